# revision 1
# baseline (speedup 1.0000x reference)
"""APPNP_Net Trainium2 kernel (8 NeuronCores, SPMD row-sharded).

The reference model is:
    h = relu(x @ W1 + b1); z = h @ W2 + b2; out = log_softmax(z, axis=1)
followed by K=10 APPNP propagation steps with ALPHA=1.0.  Since
z_{t+1} = (1-ALPHA)*agg + ALPHA*h == h, the propagation is the identity
and edge_index never affects the output.  So the kernel is a row-wise
MLP + log_softmax, sharded by nodes across the 8 cores.

Device layout choices:
  - x is pre-transposed on the host to xT [512, rows] so features land on
    SBUF partitions (matmul contracts along partitions); no on-device
    transpose needed.
  - bf16 matmul operands (x, W1, h, W2), fp32 PSUM accumulation and fp32
    softmax/output.  bf16 halves the DMA traffic and runs the PE at full
    rate (fp32 matmul is 4x slower).
  - MM1 computes hT [hid, rows] (lhsT = W1 slices); MM2 consumes hT
    directly as the stationary operand producing z [rows, 50], which is
    the natural layout for the free-dim log_softmax and the output DMA.
"""

import sys

sys.path.insert(0, "/opt/trn_rl_repo")

import numpy as np
import ml_dtypes

import concourse.tile as tile
from concourse import bacc, mybir
from concourse.bass_utils import run_bass_kernel_spmd

N_NODES = 100000
F_IN = 512
HID = 256
C = 50
N_CORES = 8
ROWS_PER_CORE = 12544  # 98 * 128; 8 * 12544 = 100352 >= 100000 (zero-padded)
BLOCK = 512
KC = F_IN // 128  # 4 contraction chunks for MM1
MH = HID // 128  # 2 hidden chunks

BF16 = mybir.dt.bfloat16
F32 = mybir.dt.float32
np_bf16 = ml_dtypes.bfloat16


def build_nc():
    nc = bacc.Bacc(
        "TRN2",
        target_bir_lowering=False,
        debug=False,
        num_devices=N_CORES,
    )
    xT = nc.declare_dram_parameter("xT", [F_IN, ROWS_PER_CORE], BF16, isOutput=False).ap()
    W1 = nc.declare_dram_parameter("W1", [F_IN, HID], BF16, isOutput=False).ap()
    W2 = nc.declare_dram_parameter("W2", [HID, C], BF16, isOutput=False).ap()
    b1 = nc.declare_dram_parameter("b1", [128, MH], F32, isOutput=False).ap()
    b2b = nc.declare_dram_parameter("b2b", [128, 4, C], F32, isOutput=False).ap()
    out = nc.declare_dram_parameter("out", [ROWS_PER_CORE, C], F32, isOutput=True).ap()

    # out rows are q*128 + p; view as [p, q, c] so the SBUF partition dim
    # maps to p and one 3-dim DMA stores a whole block.
    out_v = out.rearrange("(q p) c -> p q c", p=128)

    blocks = []
    r = 0
    while r < ROWS_PER_CORE:
        R = min(BLOCK, ROWS_PER_CORE - r)
        blocks.append((r, R))
        r += R

    from contextlib import ExitStack

    with tile.TileContext(nc) as tc, ExitStack() as ctx:
        consts = ctx.enter_context(tc.tile_pool(name="consts", bufs=1))
        xpool = ctx.enter_context(tc.tile_pool(name="x", bufs=3))
        hpool = ctx.enter_context(tc.tile_pool(name="h", bufs=2))
        spool = ctx.enter_context(tc.tile_pool(name="s", bufs=2))
        opool = ctx.enter_context(tc.tile_pool(name="o", bufs=2))
        psum = ctx.enter_context(tc.tile_pool(name="psum", bufs=2, space="PSUM"))

        w1sb = []
        for k in range(KC):
            t = consts.tile([128, HID], BF16, tag=f"w1_{k}")
            nc.sync.dma_start(out=t, in_=W1[k * 128 : (k + 1) * 128, :])
            w1sb.append(t)
        w2sb = []
        for kh in range(MH):
            t = consts.tile([128, C], BF16, tag=f"w2_{kh}")
            nc.sync.dma_start(out=t, in_=W2[kh * 128 : (kh + 1) * 128, :])
            w2sb.append(t)
        b1sb = consts.tile([128, MH], F32, tag="b1")
        nc.sync.dma_start(out=b1sb, in_=b1)
        b2sb = consts.tile([128, 4, C], F32, tag="b2")
        nc.sync.dma_start(out=b2sb, in_=b2b)

        for r0, R in blocks:
            S = R // 128
            xts = []
            for k in range(KC):
                xt = xpool.tile([128, BLOCK], BF16, tag=f"xt{k}")
                nc.sync.dma_start(
                    out=xt[:, :R], in_=xT[k * 128 : (k + 1) * 128, r0 : r0 + R]
                )
                xts.append(xt)

            hs = []
            for mh in range(MH):
                ph = psum.tile([128, BLOCK], F32, tag=f"ph{mh}")
                for k in range(KC):
                    nc.tensor.matmul(
                        ph[:, :R],
                        lhsT=w1sb[k][:, mh * 128 : (mh + 1) * 128],
                        rhs=xts[k][:, :R],
                        start=(k == 0),
                        stop=(k == KC - 1),
                    )
                h = hpool.tile([128, BLOCK], BF16, tag=f"h{mh}")
                nc.scalar.activation(
                    h[:, :R],
                    ph[:, :R],
                    mybir.ActivationFunctionType.Relu,
                    bias=b1sb[:, mh : mh + 1],
                )
                hs.append(h)

            pz = psum.tile([128, 4, C], F32, tag="pz")
            for rs in range(S):
                for kh in range(MH):
                    nc.tensor.matmul(
                        pz[:, rs, :],
                        lhsT=hs[kh][:, rs * 128 : (rs + 1) * 128],
                        rhs=w2sb[kh],
                        start=(kh == 0),
                        stop=(kh == MH - 1),
                    )

            # z = pz + b2 ; out = z - log(sum(exp(z)))   (|z| is O(1): no
            # max-subtraction needed for fp32 exp)
            tt = opool.tile([128, 4, C], F32, tag="t")
            nc.vector.tensor_add(tt[:, :S, :], pz[:, :S, :], b2sb[:, :S, :])
            s4 = spool.tile([128, 4], F32, tag="s4")
            e = spool.tile([128, C], F32, tag="e")
            for rs in range(S):
                nc.scalar.activation(
                    e,
                    tt[:, rs, :],
                    mybir.ActivationFunctionType.Exp,
                    accum_out=s4[:, rs : rs + 1],
                )
            ls4 = spool.tile([128, 4], F32, tag="ls4")
            nc.scalar.activation(
                ls4[:, :S], s4[:, :S], mybir.ActivationFunctionType.Ln
            )
            zo = opool.tile([128, 4, C], F32, tag="zo")
            for rs in range(S):
                nc.vector.tensor_scalar(
                    out=zo[:, rs, :],
                    in0=tt[:, rs, :],
                    scalar1=ls4[:, rs : rs + 1],
                    scalar2=None,
                    op0=mybir.AluOpType.subtract,
                )
            q0 = r0 // 128
            nc.sync.dma_start(out=out_v[:, q0 : q0 + S, :], in_=zo[:, :S, :])

    nc.compile()
    return nc


_NC = None


def _get_nc():
    global _NC
    if _NC is None:
        _NC = build_nc()
    return _NC


def make_in_maps(x, W1, b1, W2, b2):
    x = np.asarray(x, dtype=np.float32)
    W1bf = np.asarray(W1, dtype=np.float32).astype(np_bf16)
    W2bf = np.asarray(W2, dtype=np.float32).astype(np_bf16)
    # b1 laid out [p, m] with b1[m*128 + p] so column m is the per-partition
    # bias for hidden chunk m.
    b1t = np.ascontiguousarray(
        np.asarray(b1, dtype=np.float32).reshape(MH, 128).T
    )
    b2b = np.tile(np.asarray(b2, dtype=np.float32), (128, 4)).reshape(128, 4, C)
    b2b = np.ascontiguousarray(b2b)

    in_maps = []
    for i in range(N_CORES):
        r0 = i * ROWS_PER_CORE
        r1 = min(r0 + ROWS_PER_CORE, N_NODES)
        shard = x[r0:r1].astype(np_bf16)
        if shard.shape[0] < ROWS_PER_CORE:
            pad = np.zeros((ROWS_PER_CORE - shard.shape[0], F_IN), dtype=np_bf16)
            shard = np.concatenate([shard, pad], axis=0)
        xT = np.ascontiguousarray(shard.T)
        in_maps.append({"xT": xT, "W1": W1bf, "W2": W2bf, "b1": b1t, "b2b": b2b})
    return in_maps


def run(x, W1, b1, W2, b2, trace=False, **spmd_kwargs):
    nc = _get_nc()
    in_maps = make_in_maps(x, W1, b1, W2, b2)
    res = run_bass_kernel_spmd(
        nc, in_maps, core_ids=list(range(N_CORES)), trace=trace, **spmd_kwargs
    )
    outs = [np.asarray(res.results[i]["out"]) for i in range(N_CORES)]
    full = np.concatenate(outs, axis=0)[:N_NODES]
    return full.astype(np.float32, copy=False), res


def kernel(x, edge_index, W1, b1, W2, b2):
    out, _ = run(x, W1, b1, W2, b2, trace=False)
    return out


# revision 2
# speedup vs baseline: 1.7493x; 1.7493x over previous
"""APPNP_Net Trainium2 kernel (8 NeuronCores, SPMD row-sharded).

The reference model is:
    h = relu(x @ W1 + b1); z = h @ W2 + b2; out = log_softmax(z, axis=1)
followed by K=10 APPNP propagation steps with ALPHA=1.0.  Since
z_{t+1} = (1-ALPHA)*agg + ALPHA*h == h, the propagation is the identity
and edge_index never affects the output.  So the kernel is a row-wise
MLP + log_softmax, sharded by nodes across the 8 cores.

Device layout choices:
  - x is pre-transposed + pre-tiled on the host to [25, 4, 128, 512]
    (block, k-chunk, feature-partition, row) so features land on SBUF
    partitions (matmul contracts along partitions) and every x DMA is a
    fully contiguous 128 KiB chunk.
  - bf16 matmul operands (x, W1, h, W2), fp32 PSUM accumulation and
    fp32 softmax/output.  bf16 halves the DMA traffic and runs the PE
    at full rate (fp32 matmul is 4x slower).
  - MM1 computes hT [hid, rows] (lhsT = W1 slices); MM2 consumes hT
    directly as the stationary operand producing z [rows, 50], the
    natural layout for the free-dim log_softmax and the output DMA.
  - ScalarE activation-table discipline: the steady loop only uses
    Relu/Exp (one table set).  Ln is batched per group of 5 blocks so
    the ~1.5us ACT_TABLE_LOAD set switch happens 2x per group instead
    of 2x per block.
  - log-softmax skips the max-subtraction: z = h@W2+b2 is O(1) here,
    exp is safe in fp32.
  - engine balance per block: PE MM1+MM2; ACT relu(mh0)+exp; DVE
    relu(mh1)+bias-add+reduce+broadcast-subtract.
"""

import sys

sys.path.insert(0, "/opt/trn_rl_repo")

import dataclasses
from contextlib import ExitStack

import numpy as np
import ml_dtypes

import concourse.tile as tile
from concourse import bacc, mybir
from concourse.bass_utils import run_bass_kernel_spmd

N_NODES = 100000
F_IN = 512
HID = 256
C = 50
N_CORES = 8
ROWS_PER_CORE = 12800  # 25 blocks x 512; 8*12800 = 102400 >= 100000 (zero-padded)
BLOCK = 512
NBLK = ROWS_PER_CORE // BLOCK  # 25
GROUP = 5  # blocks per log-softmax group (Ln batching)
KC = F_IN // 128  # 4 contraction chunks for MM1
MH = HID // 128  # 2 hidden chunks
SUB = BLOCK // 128  # 4 row-subtiles per block

BF16 = mybir.dt.bfloat16
F32 = mybir.dt.float32
np_bf16 = ml_dtypes.bfloat16


def _bcast_cols(ap2d, reps):
    """[P, Q] AP -> [P, Q, reps] AP with a zero-stride inner dim."""
    return dataclasses.replace(ap2d, ap=[ap2d.ap[0], ap2d.ap[1], [0, reps]])


def build_nc():
    nc = bacc.Bacc(
        "TRN2",
        target_bir_lowering=False,
        debug=False,
        num_devices=N_CORES,
    )
    xT = nc.declare_dram_parameter(
        "xT", [NBLK, KC, 128, BLOCK], BF16, isOutput=False
    ).ap()
    W1 = nc.declare_dram_parameter("W1", [F_IN, HID], BF16, isOutput=False).ap()
    W2 = nc.declare_dram_parameter("W2", [HID, C], BF16, isOutput=False).ap()
    b1 = nc.declare_dram_parameter("b1", [128, MH], F32, isOutput=False).ap()
    b2b = nc.declare_dram_parameter("b2b", [128, SUB, C], F32, isOutput=False).ap()
    # out[p, q, c] with row = q*128 + p (host transposes back); per-partition
    # contiguous runs keep the store DMAs cheap.
    out = nc.declare_dram_parameter(
        "out", [128, NBLK * SUB, C], F32, isOutput=True
    ).ap()

    with tile.TileContext(nc) as tc, ExitStack() as ctx:
        consts = ctx.enter_context(tc.tile_pool(name="consts", bufs=1))
        xpool = ctx.enter_context(tc.tile_pool(name="x", bufs=3))
        hpool = ctx.enter_context(tc.tile_pool(name="h", bufs=2))
        tpool = ctx.enter_context(tc.tile_pool(name="t", bufs=2))
        epool = ctx.enter_context(tc.tile_pool(name="e", bufs=2))
        spool = ctx.enter_context(tc.tile_pool(name="s", bufs=2))
        opool = ctx.enter_context(tc.tile_pool(name="o", bufs=3))
        psum = ctx.enter_context(tc.tile_pool(name="psum", bufs=2, space="PSUM"))

        w1sb = []
        for k in range(KC):
            t = consts.tile([128, HID], BF16, tag=f"w1_{k}")
            nc.sync.dma_start(out=t, in_=W1[k * 128 : (k + 1) * 128, :])
            w1sb.append(t)
        w2sb = []
        for kh in range(MH):
            t = consts.tile([128, C], BF16, tag=f"w2_{kh}")
            nc.sync.dma_start(out=t, in_=W2[kh * 128 : (kh + 1) * 128, :])
            w2sb.append(t)
        b1sb = consts.tile([128, MH], F32, tag="b1")
        nc.sync.dma_start(out=b1sb, in_=b1)
        b2sb = consts.tile([128, SUB, C], F32, tag="b2")
        nc.sync.dma_start(out=b2sb, in_=b2b)

        for g in range(NBLK // GROUP):
            t_g = tpool.tile([128, GROUP, SUB, C], F32, tag="t")
            s_g = spool.tile([128, GROUP * SUB], F32, tag="s")
            for j in range(GROUP):
                b = g * GROUP + j
                xts = []
                for k in range(KC):
                    xt = xpool.tile([128, BLOCK], BF16, tag=f"xt{k}")
                    nc.sync.dma_start(out=xt, in_=xT[b, k])
                    xts.append(xt)

                hs = []
                for mh in range(MH):
                    ph = psum.tile([128, BLOCK], F32, tag=f"ph{mh}")
                    for k in range(KC):
                        nc.tensor.matmul(
                            ph,
                            lhsT=w1sb[k][:, mh * 128 : (mh + 1) * 128],
                            rhs=xts[k],
                            start=(k == 0),
                            stop=(k == KC - 1),
                        )
                    h = hpool.tile([128, BLOCK], BF16, tag=f"h{mh}")
                    if mh == 0:
                        # ScalarE: relu(ph + b1)
                        nc.scalar.activation(
                            h,
                            ph,
                            mybir.ActivationFunctionType.Relu,
                            bias=b1sb[:, mh : mh + 1],
                        )
                    else:
                        # VectorE: (ph + b1) max 0 — balance the engines
                        nc.vector.tensor_scalar(
                            out=h,
                            in0=ph,
                            scalar1=b1sb[:, mh : mh + 1],
                            scalar2=0.0,
                            op0=mybir.AluOpType.add,
                            op1=mybir.AluOpType.max,
                        )
                    hs.append(h)

                pz = psum.tile([128, SUB, C], F32, tag="pz")
                for rs in range(SUB):
                    for kh in range(MH):
                        nc.tensor.matmul(
                            pz[:, rs, :],
                            lhsT=hs[kh][:, rs * 128 : (rs + 1) * 128],
                            rhs=w2sb[kh],
                            start=(kh == 0),
                            stop=(kh == MH - 1),
                        )

                # t = z = pz + b2 (also moves PSUM -> SBUF for the tail)
                nc.vector.tensor_add(t_g[:, j], pz, b2sb)
                # e = exp(z); s = rowsum(e) per 50-class group
                e = epool.tile([128, SUB, C], F32, tag="e")
                nc.scalar.activation(e, t_g[:, j], mybir.ActivationFunctionType.Exp)
                nc.vector.reduce_sum(
                    out=s_g[:, j * SUB : (j + 1) * SUB],
                    in_=e,
                    axis=mybir.AxisListType.X,
                )

            # group tail: ls = ln(s); out = z - ls (broadcast over classes)
            ls_g = spool.tile([128, GROUP * SUB], F32, tag="ls")
            nc.scalar.activation(ls_g, s_g, mybir.ActivationFunctionType.Ln)
            for j in range(GROUP):
                b = g * GROUP + j
                zo = opool.tile([128, SUB, C], F32, tag="zo")
                ls_cols = ls_g[:, j * SUB : (j + 1) * SUB]
                nc.vector.tensor_tensor(
                    out=zo,
                    in0=t_g[:, j],
                    in1=_bcast_cols(ls_cols, C),
                    op=mybir.AluOpType.subtract,
                )
                q0 = b * SUB
                nc.sync.dma_start(out=out[:, q0 : q0 + SUB, :], in_=zo)

    nc.compile()
    return nc


_NC = None


def _get_nc():
    global _NC
    if _NC is None:
        _NC = build_nc()
    return _NC


def make_in_maps(x, W1, b1, W2, b2):
    x = np.asarray(x, dtype=np.float32)
    W1bf = np.asarray(W1, dtype=np.float32).astype(np_bf16)
    W2bf = np.asarray(W2, dtype=np.float32).astype(np_bf16)
    # b1 laid out [p, m]: column m is the per-partition bias of hidden chunk m
    b1t = np.ascontiguousarray(np.asarray(b1, dtype=np.float32).reshape(MH, 128).T)
    b2b = np.ascontiguousarray(
        np.tile(np.asarray(b2, dtype=np.float32), (128, SUB)).reshape(128, SUB, C)
    )

    in_maps = []
    for i in range(N_CORES):
        r0 = i * ROWS_PER_CORE
        r1 = min(r0 + ROWS_PER_CORE, N_NODES)
        shard = np.zeros((ROWS_PER_CORE, F_IN), dtype=np_bf16)
        shard[: r1 - r0] = x[r0:r1].astype(np_bf16)
        # [rows, feat] -> [block, k, p, r]
        xt = np.ascontiguousarray(
            shard.reshape(NBLK, BLOCK, KC, 128).transpose(0, 2, 3, 1)
        )
        in_maps.append({"xT": xt, "W1": W1bf, "W2": W2bf, "b1": b1t, "b2b": b2b})
    return in_maps


def run(x, W1, b1, W2, b2, trace=False, **spmd_kwargs):
    nc = _get_nc()
    in_maps = make_in_maps(x, W1, b1, W2, b2)
    res = run_bass_kernel_spmd(
        nc, in_maps, core_ids=list(range(N_CORES)), trace=trace, **spmd_kwargs
    )
    outs = []
    for i in range(N_CORES):
        o = np.asarray(res.results[i]["out"])  # [128, 100, 50], row = q*128+p
        outs.append(o.transpose(1, 0, 2).reshape(ROWS_PER_CORE, C))
    full = np.concatenate(outs, axis=0)[:N_NODES]
    return np.ascontiguousarray(full.astype(np.float32, copy=False)), res


def kernel(x, edge_index, W1, b1, W2, b2):
    out, _ = run(x, W1, b1, W2, b2, trace=False)
    return out


# revision 4
# speedup vs baseline: 2.1956x; 1.2551x over previous
"""APPNP_Net Trainium2 kernel (8 NeuronCores, SPMD row-sharded).

The reference model is:
    h = relu(x @ W1 + b1); z = h @ W2 + b2; out = log_softmax(z, axis=1)
followed by K=10 APPNP propagation steps with ALPHA=1.0.  Since
z_{t+1} = (1-ALPHA)*agg + ALPHA*h == h, the propagation is the identity
and edge_index never affects the output.  So the kernel is a row-wise
MLP + log_softmax, sharded by nodes across the 8 cores.

Device design:
  - x pre-transposed + pre-tiled on host to [13, 4, 128, 1024]
    (block-pair, k-chunk, feature-partition, row): features on SBUF
    partitions (matmul contracts along partitions); every x DMA is one
    fully contiguous 256 KiB transfer (2 KiB per partition) to keep
    descriptor counts and trigger counts low.
  - bf16 matmul operands (x, W1, h, W2), fp32 PSUM/softmax/output.
  - MM1 makes hT [hid, rows] (lhsT = W1 slices); MM2 consumes hT as the
    stationary operand producing z [rows, 50] — the right layout for
    free-dim log_softmax and the output DMA.
  - PE software pipeline: MM2(b-1) is emitted between MM1(b) and
    MM1(b+1), so the PE never waits on the relu producing h(b).
  - ScalarE table discipline: steady loop uses only Relu/Exp (one ACT
    table set); Ln batched once per 13-block group.
  - engine balance: ACT relu(mh0)+exp; DVE relu(mh1)+bias-add+reduce+
    broadcast-subtract (stride-0 AP broadcast of the log-sum).
  - log-softmax skips max-subtraction (z is O(1); fp32 exp is safe).
"""

import sys

sys.path.insert(0, "/opt/trn_rl_repo")

import dataclasses
from contextlib import ExitStack

import numpy as np
import ml_dtypes

import concourse.tile as tile
from concourse import bacc, mybir
from concourse.bass_utils import run_bass_kernel_spmd

N_NODES = 100000
F_IN = 512
HID = 256
C = 50
N_CORES = 8
BLOCK = 512
NBLK = 26  # blocks per core
NPAIR = NBLK // 2
ROWS_PER_CORE = NBLK * BLOCK  # 13312; 8*13312 = 106496 >= 100000 (zero-padded)
GROUP = 13  # blocks per log-softmax group (Ln batching)
KC = F_IN // 128  # 4 contraction chunks for MM1
MH = HID // 128  # 2 hidden chunks
SUB = BLOCK // 128  # 4 row-subtiles per block

BF16 = mybir.dt.bfloat16
F32 = mybir.dt.float32
np_bf16 = ml_dtypes.bfloat16


def _bcast_cols(ap2d, reps):
    """[P, Q] AP -> [P, Q, reps] AP with a zero-stride inner dim."""
    return dataclasses.replace(ap2d, ap=[ap2d.ap[0], ap2d.ap[1], [0, reps]])


def build_nc():
    nc = bacc.Bacc(
        "TRN2",
        target_bir_lowering=False,
        debug=False,
        num_devices=N_CORES,
    )
    xT = nc.declare_dram_parameter(
        "xT", [NPAIR, KC, 128, 2 * BLOCK], BF16, isOutput=False
    ).ap()
    W1 = nc.declare_dram_parameter("W1", [F_IN, HID], BF16, isOutput=False).ap()
    W2 = nc.declare_dram_parameter("W2", [HID, C], BF16, isOutput=False).ap()
    b1 = nc.declare_dram_parameter("b1", [128, MH], F32, isOutput=False).ap()
    b2b = nc.declare_dram_parameter("b2b", [128, SUB, C], F32, isOutput=False).ap()
    # out[p, q, c] with row = q*128 + p (host transposes back); per-partition
    # contiguous 800 B runs keep the store DMAs cheap.
    out = nc.declare_dram_parameter(
        "out", [128, NBLK * SUB, C], F32, isOutput=True
    ).ap()

    with tile.TileContext(nc) as tc, ExitStack() as ctx:
        consts = ctx.enter_context(tc.tile_pool(name="consts", bufs=1))
        xpool = ctx.enter_context(tc.tile_pool(name="x", bufs=3))
        hpool = ctx.enter_context(tc.tile_pool(name="h", bufs=2))
        tpool = ctx.enter_context(tc.tile_pool(name="t", bufs=2))
        epool = ctx.enter_context(tc.tile_pool(name="e", bufs=2))
        spool = ctx.enter_context(tc.tile_pool(name="s", bufs=2))
        opool = ctx.enter_context(tc.tile_pool(name="o", bufs=3))
        psum = ctx.enter_context(tc.tile_pool(name="psum", bufs=2, space="PSUM"))

        xts = {}  # pair -> [4 tiles of [128, 1024] bf16]

        def issue_pair(p):
            tiles = []
            for k in range(KC):
                xt = xpool.tile([128, 2 * BLOCK], BF16, tag=f"xt{k}")
                nc.sync.dma_start(out=xt, in_=xT[p, k])
                tiles.append(xt)
            xts[p] = tiles

        # Prefetch block pair 0 before anything else so MM1(0) starts early.
        issue_pair(0)
        w1sb = []
        for k in range(KC):
            t = consts.tile([128, HID], BF16, tag=f"w1_{k}")
            nc.sync.dma_start(out=t, in_=W1[k * 128 : (k + 1) * 128, :])
            w1sb.append(t)
        b1sb = consts.tile([128, MH], F32, tag="b1")
        nc.sync.dma_start(out=b1sb, in_=b1)
        w2sb = []
        for kh in range(MH):
            t = consts.tile([128, C], BF16, tag=f"w2_{kh}")
            nc.sync.dma_start(out=t, in_=W2[kh * 128 : (kh + 1) * 128, :])
            w2sb.append(t)
        b2sb = consts.tile([128, SUB, C], F32, tag="b2")
        nc.sync.dma_start(out=b2sb, in_=b2b)

        hs = {}  # block -> [h0, h1]
        t_gs = {}  # group -> t tile [128, GROUP, SUB, C]
        s_gs = {}  # group -> s tile [128, GROUP*SUB]

        def emit_front(b):
            """x DMA (per pair), MM1, relu for block b."""
            pair, sub = divmod(b, 2)
            if sub == 0 and pair not in xts:
                issue_pair(pair)
            xtiles = xts[pair]
            hcur = []
            for mh in range(MH):
                ph = psum.tile([128, BLOCK], F32, tag=f"ph{mh}")
                for k in range(KC):
                    nc.tensor.matmul(
                        ph,
                        lhsT=w1sb[k][:, mh * 128 : (mh + 1) * 128],
                        rhs=xtiles[k][:, sub * BLOCK : (sub + 1) * BLOCK],
                        start=(k == 0),
                        stop=(k == KC - 1),
                    )
                h = hpool.tile([128, BLOCK], BF16, tag=f"h{mh}")
                if mh == 0:
                    # ScalarE: relu(ph + b1)
                    nc.scalar.activation(
                        h,
                        ph,
                        mybir.ActivationFunctionType.Relu,
                        bias=b1sb[:, mh : mh + 1],
                    )
                else:
                    # VectorE: (ph + b1) max 0 — balance the engines
                    nc.vector.tensor_scalar(
                        out=h,
                        in0=ph,
                        scalar1=b1sb[:, mh : mh + 1],
                        scalar2=0.0,
                        op0=mybir.AluOpType.add,
                        op1=mybir.AluOpType.max,
                    )
                hcur.append(h)
            hs[b] = hcur

        def emit_back(b):
            """MM2, bias-add, exp, row-sum for block b."""
            g, j = divmod(b, GROUP)
            if j == 0:
                t_gs[g] = tpool.tile([128, GROUP, SUB, C], F32, tag="t", name=f"t_g{g}")
                s_gs[g] = spool.tile([128, GROUP * SUB], F32, tag="s", name=f"s_g{g}")
            t_g, s_g = t_gs[g], s_gs[g]
            hb = hs.pop(b)
            pz = psum.tile([128, SUB, C], F32, tag="pz")
            for rs in range(SUB):
                for kh in range(MH):
                    nc.tensor.matmul(
                        pz[:, rs, :],
                        lhsT=hb[kh][:, rs * 128 : (rs + 1) * 128],
                        rhs=w2sb[kh],
                        start=(kh == 0),
                        stop=(kh == MH - 1),
                    )
            # t = z = pz + b2 (also moves PSUM -> SBUF for the tail)
            nc.vector.tensor_add(t_g[:, j], pz, b2sb)
            e = epool.tile([128, SUB, C], F32, tag="e")
            nc.scalar.activation(e, t_g[:, j], mybir.ActivationFunctionType.Exp)
            nc.vector.reduce_sum(
                out=s_g[:, j * SUB : (j + 1) * SUB],
                in_=e,
                axis=mybir.AxisListType.X,
            )

        def emit_group_tail(g):
            """ls = ln(s); out = z - ls (class-broadcast); store."""
            t_g, s_g = t_gs.pop(g), s_gs.pop(g)
            ls_g = spool.tile([128, GROUP * SUB], F32, tag="ls")
            nc.scalar.activation(ls_g, s_g, mybir.ActivationFunctionType.Ln)
            for j in range(GROUP):
                b = g * GROUP + j
                zo = opool.tile([128, SUB, C], F32, tag="zo")
                ls_cols = ls_g[:, j * SUB : (j + 1) * SUB]
                nc.vector.tensor_tensor(
                    out=zo,
                    in0=t_g[:, j],
                    in1=_bcast_cols(ls_cols, C),
                    op=mybir.AluOpType.subtract,
                )
                q0 = b * SUB
                nc.sync.dma_start(out=out[:, q0 : q0 + SUB, :], in_=zo)

        for b in range(NBLK):
            emit_front(b)
            if b >= 1:
                emit_back(b - 1)
                if b % GROUP == 0:
                    emit_group_tail(b // GROUP - 1)
        emit_back(NBLK - 1)
        emit_group_tail(NBLK // GROUP - 1)

    nc.compile()
    return nc


_NC = None


def _get_nc():
    global _NC
    if _NC is None:
        _NC = build_nc()
    return _NC


def make_in_maps(x, W1, b1, W2, b2):
    x = np.asarray(x, dtype=np.float32)
    W1bf = np.asarray(W1, dtype=np.float32).astype(np_bf16)
    W2bf = np.asarray(W2, dtype=np.float32).astype(np_bf16)
    # b1 laid out [p, m]: column m is the per-partition bias of hidden chunk m
    b1t = np.ascontiguousarray(np.asarray(b1, dtype=np.float32).reshape(MH, 128).T)
    b2b = np.ascontiguousarray(
        np.tile(np.asarray(b2, dtype=np.float32), (128, SUB)).reshape(128, SUB, C)
    )

    in_maps = []
    for i in range(N_CORES):
        r0 = i * ROWS_PER_CORE
        r1 = min(r0 + ROWS_PER_CORE, N_NODES)
        shard = np.zeros((ROWS_PER_CORE, F_IN), dtype=np_bf16)
        shard[: r1 - r0] = x[r0:r1].astype(np_bf16)
        # [rows, feat] -> [pair, k, p, r]
        xt = np.ascontiguousarray(
            shard.reshape(NPAIR, 2 * BLOCK, KC, 128).transpose(0, 2, 3, 1)
        )
        in_maps.append({"xT": xt, "W1": W1bf, "W2": W2bf, "b1": b1t, "b2b": b2b})
    return in_maps


def run(x, W1, b1, W2, b2, trace=False, **spmd_kwargs):
    nc = _get_nc()
    in_maps = make_in_maps(x, W1, b1, W2, b2)
    res = run_bass_kernel_spmd(
        nc, in_maps, core_ids=list(range(N_CORES)), trace=trace, **spmd_kwargs
    )
    outs = []
    for i in range(N_CORES):
        o = np.asarray(res.results[i]["out"])  # [128, 104, 50], row = q*128+p
        outs.append(o.transpose(1, 0, 2).reshape(ROWS_PER_CORE, C))
    full = np.concatenate(outs, axis=0)[:N_NODES]
    return np.ascontiguousarray(full.astype(np.float32, copy=False)), res


def kernel(x, edge_index, W1, b1, W2, b2):
    out, _ = run(x, W1, b1, W2, b2, trace=False)
    return out
